# revision 3
# baseline (speedup 1.0000x reference)
"""CPSF codebook fused kernel for 8 Trainium2 NeuronCores.

Math (see reference): for each batch row b and codebook entry m,
  q[b,m] = par_sq/s_par + (tot_sq-par_sq + dd_sq)/s_perp
  w[b,m] = alpha[m] * exp(-pi*q)
  out    = Re((w @ (T_hat_re + i*T_hat_im)) @ A.T),  A = exp(i*2pi/S * k*s)

Device strategy (pure batch-parallel, no collectives):
  - DFT folded into the codebook on host: out = w @ TA,
    TA = T_hat_re @ cos(ang) - T_hat_im @ sin(ang).
  - All per-(b,m) math is expressed so the device computes
        w[m,b] = exp(g1^2 + g2^2 + nb + be[m]),
    where g1,g2,nb are matmul outputs. The per-m shifts of the parallel
    component are linear in the batch vector, so they fold into the nb
    matmul weights; sqrt(pi*(s_perp/s_par-1)/s_perp) folds into the
    g-matmul weights. The squares are then bias- and scale-free, and the
    per-b |z|^2 term folds into an output row scale (uniform sigma).
  - Precision (validated vs fp64 reference, ~4e-3 max rel err): matmul
    operands bf16 (optionally fp8 e4m3 + DoubleRow for the g/dd matmuls),
    fp32 PSUM accumulation, fp16 squares, bf16 weights w.
  - Each core handles B/8 = 512 batch rows against the full codebook.
"""

import os
import sys

for _p in ("/opt/trn_rl_repo", os.path.expanduser("~/.axon_site/_ro/trn_rl_repo")):
    if os.path.isdir(_p) and _p not in sys.path:
        sys.path.insert(0, _p)

import numpy as np

B, N, M, S = 4096, 64, 8192, 256
NCORES = 8
BLOC = B // NCORES          # 512 batch rows per core
NT = M // 128               # 64 codebook tiles
PI = float(np.pi)

USE_FP8 = False             # fp8e4m3+DoubleRow for g1/g2 and vd~vd_j matmuls
THETA = 0                   # square cols (of 1024) via DVE-copy + Pool-square
POOL_ADD = 0                # cols (of 512) of the fp16 pair-add done on Pool


def _f8(x):
    import ml_dtypes

    return np.ascontiguousarray(x.astype(np.float32).astype(ml_dtypes.float8_e4m3fn))


def _bf(x):
    import ml_dtypes

    return np.ascontiguousarray(x.astype(np.float32).astype(ml_dtypes.bfloat16))


def _pair(a):
    """[128, X] -> [64, 2, X] DoubleRow layout: (k, j, x) = a[k + 64*j, x]."""
    return np.ascontiguousarray(a.reshape(2, 64, -1).transpose(1, 0, 2))


def _prep(x_re, x_im, z_j_re, z_j_im, vec_d_j_re, vec_d_j_im,
          T_hat_re, T_hat_im, alpha_j, sigma_par, sigma_perp):
    """Host-side operand packing (all O(B*N + M*N + M*S^2) — tiny vs device work)."""
    f32 = np.float32
    f64 = np.float64
    tiny = np.finfo(f32).tiny

    # ---- batch side ----
    z_re = np.ascontiguousarray(x_re[:, :N]).astype(f32)
    z_im = np.ascontiguousarray(x_im[:, :N]).astype(f32)
    vd_re = np.ascontiguousarray(x_re[:, N:]).astype(f32)
    vd_im = np.ascontiguousarray(x_im[:, N:]).astype(f32)
    nrm = np.sqrt((vd_re * vd_re + vd_im * vd_im).sum(-1, dtype=f32)).astype(f32)
    nrm = np.where(nrm == 0, f32(1.0), nrm)
    vd_re = vd_re / nrm[:, None]
    vd_im = vd_im / nrm[:, None]
    z_sq = (z_re * z_re + z_im * z_im).sum(-1, dtype=f32)
    vd_sq = (vd_re * vd_re + vd_im * vd_im).sum(-1, dtype=f32)

    r1 = np.concatenate([z_re.T, z_im.T], 0)      # [128, B]
    r2 = np.concatenate([vd_re.T, vd_im.T], 0)    # [128, B]
    vrow = (z_sq + vd_sq).astype(f32)[None, :]    # [1, B]

    # ---- codebook side ----
    djr = z_j_re.astype(f32)
    dji = z_j_im.astype(f32)
    vr = vec_d_j_re.astype(f32)
    vi = vec_d_j_im.astype(f32)
    nj = np.sqrt((vr * vr + vi * vi).sum(-1, dtype=f32)).astype(f32)
    nj = np.where(nj == 0, f32(1.0), nj)
    vr = vr / nj[:, None]
    vi = vi / nj[:, None]

    alpha = np.maximum(alpha_j.astype(f32), tiny)
    s_par = np.maximum(sigma_par.astype(f32), tiny).astype(f64)
    s_perp = np.maximum(sigma_perp.astype(f32), tiny).astype(f64)
    inv_sp = 1.0 / s_perp
    rm1 = s_perp / s_par - 1.0
    # rm1 < 0 required for the bias-free square fold; true whenever
    # s_par > s_perp (anisotropy rho > 1, as in this module's init).
    assert np.all(rm1 < 0)
    gs = np.sqrt(PI * (-rm1) * inv_sp)            # [M]

    c0_re = (vr * djr + vi * dji).sum(-1, dtype=f32).astype(f64)
    c0_im = (vr * dji - vi * djr).sum(-1, dtype=f32).astype(f64)
    z_j_sq = (djr * djr + dji * dji).sum(-1, dtype=f32).astype(f64)
    vd_j_sq = (vr * vr + vi * vi).sum(-1, dtype=f32).astype(f64)

    L1 = np.concatenate([vr.T, vi.T], 0) * gs[None, :]        # [128, M]
    L2 = np.concatenate([-vi.T, vr.T], 0) * gs[None, :]
    b1 = -(c0_re * gs)
    b2 = -(c0_im * gs)
    L3 = (np.concatenate([djr.T, dji.T], 0) * (PI * 2.0 * inv_sp)[None, :]
          + 2.0 * b1[None, :] * L1 + 2.0 * b2[None, :] * L2)  # faces r1
    L4 = np.concatenate([vr.T, vi.T], 0) * (PI * 2.0 * inv_sp)[None, :]  # faces r2

    be = (np.log(alpha.astype(f64)) - PI * (z_j_sq + vd_j_sq) * inv_sp
          + b1 * b1 + b2 * b2).astype(f32)        # [M] exp bias per m

    # uniform-sigma row-scale fold: exp(-pi*mean(inv_sp)*(|z|^2+|vd|^2)) per b
    c0v = float(inv_sp.mean())
    delta = (inv_sp - c0v).astype(f32)
    uniform = bool(np.all(delta == 0))
    erow = np.exp(-PI * c0v * (z_sq + vd_sq).astype(f64)).astype(f32)
    osc = np.ascontiguousarray(erow.reshape(NCORES, 4, 128).transpose(0, 2, 1))

    # DFT folded into the codebook (angles replicate reference fp32 exactly)
    nn = np.arange(S, dtype=f32)
    ang = f32(2.0 * PI / S) * (nn[:, None] * nn[None, :])
    cosA = np.cos(ang).astype(f32).astype(f64)
    sinA = np.sin(ang).astype(f32).astype(f64)
    TA = (T_hat_re.astype(f64) @ cosA - T_hat_im.astype(f64) @ sinA).astype(f32)

    # ---- pack per-tile operand tensors ----
    L1t = L1.reshape(128, NT, 128)
    L2t = L2.reshape(128, NT, 128)
    L4t = L4.reshape(128, NT, 128)
    if USE_FP8:
        l12 = np.empty((NT, 64, 2, 256), _f8(np.zeros(1)).dtype)
        l4p = np.empty((NT, 64, 2, 128), l12.dtype)
        for t in range(NT):
            both = np.concatenate([L1t[:, t], L2t[:, t]], 1)    # [128, 256]
            l12[t] = _f8(_pair(both))
            l4p[t] = _f8(_pair(L4t[:, t]))
        r1g = _f8(_pair(r1))                                    # [64, 2, B]
        r2g = _f8(_pair(r2))
    else:
        l12 = np.empty((NT, 128, 256), _bf(np.zeros(1)).dtype)
        l4p = np.empty((NT, 128, 128), l12.dtype)
        for t in range(NT):
            l12[t] = _bf(np.concatenate([L1t[:, t], L2t[:, t]], 1))
            l4p[t] = _bf(L4t[:, t])
        r1g = _bf(r1)
        r2g = _bf(r2)

    l3 = np.ascontiguousarray(
        _bf(L3).reshape(128, NT, 128).transpose(1, 0, 2))
    ta = np.ascontiguousarray(_bf(TA).reshape(NT, 128, S))
    bet = np.ascontiguousarray(be.reshape(NT, 128).T)           # [128, NT]
    r1b = _bf(r1)

    return dict(l12=l12, l3=l3, l4=l4p, ta=ta, bet=bet,
                r1g=r1g, r2g=r2g, r1b=r1b, osc=osc,
                vrow=vrow, delta=np.ascontiguousarray((-PI * delta)[None, :]),
                uniform=uniform)


_CACHED = {}


def _build_nc(uniform):
    key = ("nc", uniform, USE_FP8, THETA, POOL_ADD)
    if key in _CACHED:
        return _CACHED[key]
    import concourse.bacc as bacc
    import concourse.masks as masks
    import concourse.mybir as mybir
    import concourse.tile as tile

    F32 = mybir.dt.float32
    BF16 = mybir.dt.bfloat16
    FP16 = mybir.dt.float16
    F8 = mybir.dt.float8e4
    AF = mybir.ActivationFunctionType
    OP = mybir.AluOpType
    DR = mybir.MatmulPerfMode.DoubleRow

    nc = bacc.Bacc("TRN2", target_bir_lowering=False, debug=False,
                   num_devices=NCORES)

    if USE_FP8:
        d_l12 = nc.dram_tensor("l12", [NT, 64, 2, 256], F8, kind="ExternalInput").ap()
        d_l4 = nc.dram_tensor("l4", [NT, 64, 2, 128], F8, kind="ExternalInput").ap()
        d_r1g = nc.dram_tensor("r1g", [64, 2, BLOC], F8, kind="ExternalInput").ap()
        d_r2g = nc.dram_tensor("r2g", [64, 2, BLOC], F8, kind="ExternalInput").ap()
    else:
        d_l12 = nc.dram_tensor("l12", [NT, 128, 256], BF16, kind="ExternalInput").ap()
        d_l4 = nc.dram_tensor("l4", [NT, 128, 128], BF16, kind="ExternalInput").ap()
        d_r1g = nc.dram_tensor("r1g", [128, BLOC], BF16, kind="ExternalInput").ap()
        d_r2g = nc.dram_tensor("r2g", [128, BLOC], BF16, kind="ExternalInput").ap()
    d_l3 = nc.dram_tensor("l3", [NT, 128, 128], BF16, kind="ExternalInput").ap()
    d_ta = nc.dram_tensor("ta", [NT, 128, S], BF16, kind="ExternalInput").ap()
    d_bet = nc.dram_tensor("bet", [128, NT], F32, kind="ExternalInput").ap()
    d_r1b = nc.dram_tensor("r1b", [128, BLOC], BF16, kind="ExternalInput").ap()
    d_osc = nc.dram_tensor("osc", [128, 4], F32, kind="ExternalInput").ap()
    d_vrow = nc.dram_tensor("vrow", [1, BLOC], F32, kind="ExternalInput").ap()
    d_delta = nc.dram_tensor("delta", [1, M], F32, kind="ExternalInput").ap()
    d_out = nc.dram_tensor("out", [BLOC, S], F32, kind="ExternalOutput").ap()

    AW = 1024 - THETA    # square columns handled by ACT
    DA = 512 - POOL_ADD  # pair-add columns handled by DVE

    with tile.TileContext(nc) as tc:
        with tc.tile_pool(name="const", bufs=1) as cp, \
             tc.tile_pool(name="lp", bufs=6) as lpool, \
             tc.tile_pool(name="g", bufs=2, space="PSUM") as gpool, \
             tc.tile_pool(name="nbp", bufs=2, space="PSUM") as npool, \
             tc.tile_pool(name="tacc", bufs=1, space="PSUM") as taccp, \
             tc.tile_pool(name="u", bufs=3) as upool, \
             tc.tile_pool(name="w", bufs=3) as wpool:

            if USE_FP8:
                r1g = cp.tile([64, 2, BLOC], F8)
                r2g = cp.tile([64, 2, BLOC], F8)
            else:
                r1g = cp.tile([128, BLOC], BF16)
                r2g = cp.tile([128, BLOC], BF16)
            nc.sync.dma_start(r1g[:], d_r1g)
            nc.sync.dma_start(r2g[:], d_r2g)
            r1b = cp.tile([128, BLOC], BF16)
            bet = cp.tile([128, NT], F32)
            osc = cp.tile([128, 4], F32)
            ident = cp.tile([128, 128], F32)
            nc.sync.dma_start(r1b[:], d_r1b)
            nc.sync.dma_start(bet[:], d_bet)
            nc.sync.dma_start(osc[:], d_osc)
            masks.make_identity(nc, ident[:])
            if not uniform:
                vrow32 = cp.tile([1, BLOC], F32)
                delta32 = cp.tile([1, M], F32)
                vrow = cp.tile([1, BLOC], BF16)
                delta = cp.tile([1, M], BF16)
                nc.sync.dma_start(vrow32[:], d_vrow)
                nc.sync.dma_start(delta32[:], d_delta)
                nc.vector.tensor_copy(vrow[:], vrow32[:])
                nc.vector.tensor_copy(delta[:], delta32[:])

            ot0 = taccp.tile([128, BLOC], F32)
            ot1 = taccp.tile([128, BLOC], F32)

            for t in range(NT):
                if USE_FP8:
                    l12 = lpool.tile([64, 2, 256], F8, tag="l12")
                    l4 = lpool.tile([64, 2, 128], F8, tag="l4")
                else:
                    l12 = lpool.tile([128, 256], BF16, tag="l12")
                    l4 = lpool.tile([128, 128], BF16, tag="l4")
                l3 = lpool.tile([128, 128], BF16, tag="l3")
                ta = lpool.tile([128, S], BF16, tag="ta")
                nc.sync.dma_start(l12[:], d_l12[t])
                nc.sync.dma_start(l4[:], d_l4[t])
                nc.sync.dma_start(l3[:], d_l3[t])
                nc.sync.dma_start(ta[:], d_ta[t])

                G = gpool.tile([128, 1024], F32, tag="G")
                nb = npool.tile([128, BLOC], F32, tag="NB")
                if USE_FP8:
                    nc.tensor.matmul(G[:, 0:512], l12[:, :, 0:128], r1g[:],
                                     start=True, stop=True, perf_mode=DR)
                    nc.tensor.matmul(G[:, 512:1024], l12[:, :, 128:256], r1g[:],
                                     start=True, stop=True, perf_mode=DR)
                    nc.tensor.matmul(nb[:], l3[:], r1b[:],
                                     start=True, stop=False)
                    nc.tensor.matmul(nb[:], l4[:], r2g[:],
                                     start=False, stop=uniform,
                                     perf_mode=DR, skip_group_check=True)
                else:
                    nc.tensor.matmul(G[:, 0:512], l12[:, 0:128], r1g[:],
                                     start=True, stop=True)
                    nc.tensor.matmul(G[:, 512:1024], l12[:, 128:256], r1g[:],
                                     start=True, stop=True)
                    nc.tensor.matmul(nb[:], l3[:], r1b[:],
                                     start=True, stop=False)
                    nc.tensor.matmul(nb[:], l4[:], r2g[:],
                                     start=False, stop=uniform,
                                     skip_group_check=True)
                if not uniform:
                    nc.tensor.matmul(nb[:], delta[:, t * 128:(t + 1) * 128],
                                     vrow[:], start=False, stop=True,
                                     skip_group_check=True)

                u = upool.tile([128, 1024], FP16, tag="u")
                nc.scalar.activation(u[:, 0:AW], G[:, 0:AW], AF.Square)
                if THETA:
                    # DVE copies the tail of G to SBUF; Pool squares it there
                    gc = upool.tile([128, THETA], F32, tag="gc")
                    nc.vector.tensor_copy(gc[:], G[:, AW:1024])
                    nc.gpsimd.tensor_mul(u[:, AW:1024], gc[:], gc[:])
                t16 = upool.tile([128, BLOC], FP16, tag="t16")
                if POOL_ADD:
                    nc.gpsimd.tensor_add(t16[:, DA:512], u[:, DA:512],
                                         u[:, 512 + DA:1024])
                if DA:
                    nc.vector.tensor_add(t16[:, 0:DA], u[:, 0:DA],
                                         u[:, 512:512 + DA])
                nc.vector.scalar_tensor_tensor(
                    nb[:], t16[:], 1.0, nb[:], op0=OP.mult, op1=OP.add)
                w = wpool.tile([128, BLOC], BF16, tag="w")
                nc.scalar.activation(w[:], nb[:], AF.Exp,
                                     bias=bet[:, t:t + 1], scale=1.0)

                for h, oth in ((0, ot0), (1, ot1)):
                    nc.tensor.matmul(oth[:], ta[:, h * 128:(h + 1) * 128],
                                     w[:], start=(t == 0), stop=(t == NT - 1),
                                     skip_group_check=True)

            # transpose out.T -> out, scaling rows by osc, then DMA out
            obs = [wpool.tile([128, S], F32, tag=f"ob{j}", bufs=1,
                              name=f"ob{j}") for j in range(4)]
            for h, oth in ((0, ot0), (1, ot1)):
                tsb = wpool.tile([128, BLOC], F32, tag="tsb")
                nc.scalar.copy(tsb[:], oth[:])
                for j in range(4):
                    pt = npool.tile([128, 128], F32, tag="NB", name=f"pt{h}{j}")
                    nc.tensor.transpose(pt[:], tsb[:, j * 128:(j + 1) * 128],
                                        ident[:])
                    nc.vector.tensor_scalar_mul(
                        obs[j][:, h * 128:(h + 1) * 128], pt[:], osc[:, j:j + 1])
            for j in range(4):
                nc.sync.dma_start(d_out[j * 128:(j + 1) * 128, :], obs[j][:])
    nc.compile()
    _CACHED[key] = nc
    return nc


def _run(inputs, trace=False):
    from concourse.bass_utils import run_bass_kernel_spmd

    prep = _prep(**inputs)
    nc = _build_nc(prep["uniform"])
    shared = {k: prep[k] for k in ("l12", "l3", "l4", "ta", "bet", "delta")}

    in_maps = []
    for c in range(NCORES):
        sl = slice(c * BLOC, (c + 1) * BLOC)
        if USE_FP8:
            r1gc = np.ascontiguousarray(prep["r1g"][:, :, sl])
            r2gc = np.ascontiguousarray(prep["r2g"][:, :, sl])
        else:
            r1gc = np.ascontiguousarray(prep["r1g"][:, sl])
            r2gc = np.ascontiguousarray(prep["r2g"][:, sl])
        in_maps.append(dict(r1g=r1gc, r2g=r2gc,
                            r1b=np.ascontiguousarray(prep["r1b"][:, sl]),
                            vrow=np.ascontiguousarray(prep["vrow"][:, sl]),
                            osc=np.ascontiguousarray(prep["osc"][c]),
                            **shared))
    res = run_bass_kernel_spmd(nc, in_maps, list(range(NCORES)), trace=trace)
    out = np.concatenate([res.results[c]["out"] for c in range(NCORES)], 0)
    return out.astype(np.float32), res


def kernel(**inputs):
    out, _ = _run(inputs, trace=False)
    return out


def _install_ntff_hook():
    """The agent image's antenv lacks axon_hooks; recreate it so trace=True
    can capture NTFF profiles via libaxon_pjrt.so (same mechanism as
    trn_agent_boot.trn_boot)."""
    import types

    try:
        from antenv.axon_hooks import get_axon_ntff_profile_hook  # noqa: F401
        return
    except ImportError:
        pass
    import contextlib
    import ctypes

    so_path = "/opt/axon/libaxon_pjrt.so"
    lib = ctypes.CDLL(so_path)
    lib.axon_start_nrt_profile.argtypes = [ctypes.POINTER(ctypes.c_int64),
                                           ctypes.c_size_t]
    lib.axon_start_nrt_profile.restype = ctypes.c_int64
    lib.axon_stop_nrt_profile.argtypes = [ctypes.c_char_p]
    lib.axon_stop_nrt_profile.restype = ctypes.c_int64

    @contextlib.contextmanager
    def _hook(output_dir, device_ids):
        import jax

        jax.devices()
        if device_ids:
            ids = (ctypes.c_int64 * len(device_ids))(*device_ids)
            rc = lib.axon_start_nrt_profile(ids, len(device_ids))
        else:
            rc = lib.axon_start_nrt_profile(None, 0)
        if rc != 0:
            raise RuntimeError(f"axon_start_nrt_profile rc={rc}")
        try:
            yield
        finally:
            n = lib.axon_stop_nrt_profile(str(output_dir).encode())
            if n < 0:
                raise RuntimeError(f"axon_stop_nrt_profile rc={n}")
            if n == 0:
                print("WARNING: NTFF capture wrote nothing (raced the execute)")

    mod = types.ModuleType("antenv.axon_hooks")
    mod.get_axon_ntff_profile_hook = lambda: _hook
    mod.set_axon_ntff_profile_hook = lambda h: None
    sys.modules["antenv.axon_hooks"] = mod
    import antenv

    antenv.axon_hooks = mod


def run_traced(inputs):
    _install_ntff_hook()
    return _run(inputs, trace=True)


# revision 9
# speedup vs baseline: 1.2978x; 1.2978x over previous
"""CPSF codebook fused kernel for 8 Trainium2 NeuronCores.

Math (see reference): for each batch row b and codebook entry m,
  q[b,m] = par_sq/s_par + (tot_sq-par_sq + dd_sq)/s_perp
  w[b,m] = alpha[m] * exp(-pi*q)
  out    = Re((w @ (T_hat_re + i*T_hat_im)) @ A.T),  A = exp(i*2pi/S * k*s)

Device strategy (pure batch-parallel, no collectives):
  - DFT folded into the codebook on host: out = w @ TA,
    TA = T_hat_re @ cos(ang) - T_hat_im @ sin(ang).
  - All per-(b,m) math is expressed so the device computes
        w[m,b] = exp(g1^2 + g2^2 + nb + be[m]),
    where g1,g2,nb are matmul outputs. The per-m shifts of the parallel
    component are linear in the batch vector, so they fold into the nb
    matmul weights; sqrt(pi*(s_perp/s_par-1)/s_perp) folds into the
    g-matmul weights. The squares are then bias- and scale-free, and the
    per-b |z|^2 term folds into an output row scale (uniform sigma).
  - Precision (validated vs fp64 reference, ~4e-3 max rel err): matmul
    operands bf16 (optionally fp8 e4m3 + DoubleRow for the g/dd matmuls),
    fp32 PSUM accumulation, fp16 squares, bf16 weights w.
  - Each core handles B/8 = 512 batch rows against the full codebook.
"""

import os
import sys

for _p in ("/opt/trn_rl_repo", os.path.expanduser("~/.axon_site/_ro/trn_rl_repo")):
    if os.path.isdir(_p) and _p not in sys.path:
        sys.path.insert(0, _p)

import numpy as np

B, N, M, S = 4096, 64, 8192, 256
NCORES = 8
BLOC = B // NCORES          # 512 batch rows per core
NT = M // 128               # 64 codebook tiles
PI = float(np.pi)

USE_FP8 = False             # fp8e4m3+DoubleRow for g1/g2 and vd~vd_j matmuls
THETA = 0                   # square cols (of 1024) via DVE-copy + Pool-square
POOL_ADD = 0                # cols (of 512) of the fp16 pair-add done on Pool
POOL_DIAG = True            # emit throwaway Pool ops to measure gpsimd rates


def _f8(x):
    import ml_dtypes

    return np.ascontiguousarray(x.astype(np.float32).astype(ml_dtypes.float8_e4m3fn))


def _bf(x):
    import ml_dtypes

    return np.ascontiguousarray(x.astype(np.float32).astype(ml_dtypes.bfloat16))


def _pair(a):
    """[128, X] -> [64, 2, X] DoubleRow layout: (k, j, x) = a[k + 64*j, x]."""
    return np.ascontiguousarray(a.reshape(2, 64, -1).transpose(1, 0, 2))


def _prep(x_re, x_im, z_j_re, z_j_im, vec_d_j_re, vec_d_j_im,
          T_hat_re, T_hat_im, alpha_j, sigma_par, sigma_perp):
    """Host-side operand packing (all O(B*N + M*N + M*S^2) — tiny vs device work)."""
    f32 = np.float32
    f64 = np.float64
    tiny = np.finfo(f32).tiny

    # ---- batch side ----
    z_re = np.ascontiguousarray(x_re[:, :N]).astype(f32)
    z_im = np.ascontiguousarray(x_im[:, :N]).astype(f32)
    vd_re = np.ascontiguousarray(x_re[:, N:]).astype(f32)
    vd_im = np.ascontiguousarray(x_im[:, N:]).astype(f32)
    nrm = np.sqrt((vd_re * vd_re + vd_im * vd_im).sum(-1, dtype=f32)).astype(f32)
    nrm = np.where(nrm == 0, f32(1.0), nrm)
    vd_re = vd_re / nrm[:, None]
    vd_im = vd_im / nrm[:, None]
    z_sq = (z_re * z_re + z_im * z_im).sum(-1, dtype=f32)
    vd_sq = (vd_re * vd_re + vd_im * vd_im).sum(-1, dtype=f32)

    r1 = np.concatenate([z_re.T, z_im.T], 0)      # [128, B]
    r2 = np.concatenate([vd_re.T, vd_im.T], 0)    # [128, B]
    vrow = (z_sq + vd_sq).astype(f32)[None, :]    # [1, B]

    # ---- codebook side ----
    djr = z_j_re.astype(f32)
    dji = z_j_im.astype(f32)
    vr = vec_d_j_re.astype(f32)
    vi = vec_d_j_im.astype(f32)
    nj = np.sqrt((vr * vr + vi * vi).sum(-1, dtype=f32)).astype(f32)
    nj = np.where(nj == 0, f32(1.0), nj)
    vr = vr / nj[:, None]
    vi = vi / nj[:, None]

    alpha = np.maximum(alpha_j.astype(f32), tiny)
    s_par = np.maximum(sigma_par.astype(f32), tiny).astype(f64)
    s_perp = np.maximum(sigma_perp.astype(f32), tiny).astype(f64)
    inv_sp = 1.0 / s_perp
    rm1 = s_perp / s_par - 1.0
    # rm1 < 0 required for the bias-free square fold; true whenever
    # s_par > s_perp (anisotropy rho > 1, as in this module's init).
    assert np.all(rm1 < 0)
    gs = np.sqrt(PI * (-rm1) * inv_sp)            # [M]

    c0_re = (vr * djr + vi * dji).sum(-1, dtype=f32).astype(f64)
    c0_im = (vr * dji - vi * djr).sum(-1, dtype=f32).astype(f64)
    z_j_sq = (djr * djr + dji * dji).sum(-1, dtype=f32).astype(f64)
    vd_j_sq = (vr * vr + vi * vi).sum(-1, dtype=f32).astype(f64)

    L1 = np.concatenate([vr.T, vi.T], 0) * gs[None, :]        # [128, M]
    L2 = np.concatenate([-vi.T, vr.T], 0) * gs[None, :]
    b1 = -(c0_re * gs)
    b2 = -(c0_im * gs)
    L3 = (np.concatenate([djr.T, dji.T], 0) * (PI * 2.0 * inv_sp)[None, :]
          + 2.0 * b1[None, :] * L1 + 2.0 * b2[None, :] * L2)  # faces r1
    L4 = np.concatenate([vr.T, vi.T], 0) * (PI * 2.0 * inv_sp)[None, :]  # faces r2

    be = (np.log(alpha.astype(f64)) - PI * (z_j_sq + vd_j_sq) * inv_sp
          + b1 * b1 + b2 * b2).astype(f32)        # [M] exp bias per m

    # uniform-sigma row-scale fold: exp(-pi*mean(inv_sp)*(|z|^2+|vd|^2)) per b
    c0v = float(inv_sp.mean())
    delta = (inv_sp - c0v).astype(f32)
    uniform = bool(np.all(delta == 0))
    erow = np.exp(-PI * c0v * (z_sq + vd_sq).astype(f64)).astype(f32)
    osc = np.ascontiguousarray(erow.reshape(NCORES, 4, 128).transpose(0, 2, 1))

    # DFT folded into the codebook (angles replicate reference fp32 exactly)
    nn = np.arange(S, dtype=f32)
    ang = f32(2.0 * PI / S) * (nn[:, None] * nn[None, :])
    cosA = np.cos(ang).astype(f32).astype(f64)
    sinA = np.sin(ang).astype(f32).astype(f64)
    TA = (T_hat_re.astype(f64) @ cosA - T_hat_im.astype(f64) @ sinA).astype(f32)

    # ---- pack per-tile operand tensors ----
    L1t = L1.reshape(128, NT, 128)
    L2t = L2.reshape(128, NT, 128)
    L4t = L4.reshape(128, NT, 128)
    L3t = L3.reshape(128, NT, 128)
    TAt = TA.reshape(NT, 128, S)
    if USE_FP8:
        # fp8 DoubleRow pair layout for L1/L2/L4 + bf16 L3/TA, packed per
        # tile into one [128, 1280]-byte-equivalent DMA: here kept separate
        lg = np.empty((NT, 64, 2, 384), _f8(np.zeros(1)).dtype)
        for t in range(NT):
            both = np.concatenate([L1t[:, t], L2t[:, t], L4t[:, t]], 1)
            lg[t] = _f8(_pair(both))                            # [64,2,384]
        lpack = np.empty((NT, 128, 384), _bf(np.zeros(1)).dtype)
        for t in range(NT):
            lpack[t, :, 0:128] = _bf(L3t[:, t])
            lpack[t, :, 128:384] = _bf(TAt[t])
        r1g = _f8(_pair(r1))                                    # [64, 2, B]
        r2g = _f8(_pair(r2))
    else:
        lg = np.zeros((1, 1, 1, 1), _f8(np.zeros(1)).dtype)
        lpack = np.empty((NT, 128, 768), _bf(np.zeros(1)).dtype)
        for t in range(NT):
            lpack[t, :, 0:256] = _bf(
                np.concatenate([L1t[:, t], L2t[:, t]], 1))
            lpack[t, :, 256:384] = _bf(L3t[:, t])
            lpack[t, :, 384:512] = _bf(L4t[:, t])
            lpack[t, :, 512:768] = _bf(TAt[t])
        r1g = _bf(r1)
        r2g = _bf(r2)

    bet = np.ascontiguousarray(be.reshape(NT, 128).T)           # [128, NT]
    r1b = _bf(r1)

    return dict(lpack=lpack, lg=lg, bet=bet,
                r1g=r1g, r2g=r2g, r1b=r1b, osc=osc,
                vrow=vrow, delta=np.ascontiguousarray((-PI * delta)[None, :]),
                uniform=uniform)


_CACHED = {}


def _build_nc(uniform):
    key = ("nc", uniform, USE_FP8, THETA, POOL_ADD)
    if key in _CACHED:
        return _CACHED[key]
    import concourse.bacc as bacc
    import concourse.masks as masks
    import concourse.mybir as mybir
    import concourse.tile as tile

    F32 = mybir.dt.float32
    BF16 = mybir.dt.bfloat16
    FP16 = mybir.dt.float16
    F8 = mybir.dt.float8e4
    AF = mybir.ActivationFunctionType
    OP = mybir.AluOpType
    DR = mybir.MatmulPerfMode.DoubleRow

    nc = bacc.Bacc("TRN2", target_bir_lowering=False, debug=False,
                   num_devices=NCORES)

    if USE_FP8:
        d_lg = nc.dram_tensor("lg", [NT, 64, 2, 384], F8, kind="ExternalInput").ap()
        d_lp = nc.dram_tensor("lpack", [NT, 128, 384], BF16, kind="ExternalInput").ap()
        d_r1g = nc.dram_tensor("r1g", [64, 2, BLOC], F8, kind="ExternalInput").ap()
        d_r2g = nc.dram_tensor("r2g", [64, 2, BLOC], F8, kind="ExternalInput").ap()
    else:
        d_lg = nc.dram_tensor("lg", [1, 1, 1, 1], F8, kind="ExternalInput").ap()
        d_lp = nc.dram_tensor("lpack", [NT, 128, 768], BF16, kind="ExternalInput").ap()
        d_r1g = nc.dram_tensor("r1g", [128, BLOC], BF16, kind="ExternalInput").ap()
        d_r2g = nc.dram_tensor("r2g", [128, BLOC], BF16, kind="ExternalInput").ap()
    d_bet = nc.dram_tensor("bet", [128, NT], F32, kind="ExternalInput").ap()
    d_r1b = nc.dram_tensor("r1b", [128, BLOC], BF16, kind="ExternalInput").ap()
    d_osc = nc.dram_tensor("osc", [128, 4], F32, kind="ExternalInput").ap()
    d_vrow = nc.dram_tensor("vrow", [1, BLOC], F32, kind="ExternalInput").ap()
    d_delta = nc.dram_tensor("delta", [1, M], F32, kind="ExternalInput").ap()
    d_out = nc.dram_tensor("out", [BLOC, S], F32, kind="ExternalOutput").ap()

    AW = 1024 - THETA    # square columns handled by ACT
    DA = 512 - POOL_ADD  # pair-add columns handled by DVE

    with tile.TileContext(nc) as tc:
        with tc.tile_pool(name="const", bufs=1) as cp, \
             tc.tile_pool(name="lp", bufs=6) as lpool, \
             tc.tile_pool(name="g", bufs=2, space="PSUM") as gpool, \
             tc.tile_pool(name="nbp", bufs=2, space="PSUM") as npool, \
             tc.tile_pool(name="tacc", bufs=1, space="PSUM") as taccp, \
             tc.tile_pool(name="u", bufs=3) as upool, \
             tc.tile_pool(name="w", bufs=3) as wpool:

            if USE_FP8:
                r1g = cp.tile([64, 2, BLOC], F8)
                r2g = cp.tile([64, 2, BLOC], F8)
            else:
                r1g = cp.tile([128, BLOC], BF16)
                r2g = cp.tile([128, BLOC], BF16)
            nc.sync.dma_start(r1g[:], d_r1g)
            nc.sync.dma_start(r2g[:], d_r2g)
            r1b = cp.tile([128, BLOC], BF16)
            bet = cp.tile([128, NT], F32)
            osc = cp.tile([128, 4], F32)
            ident = cp.tile([128, 128], F32)
            nc.sync.dma_start(r1b[:], d_r1b)
            nc.sync.dma_start(bet[:], d_bet)
            nc.sync.dma_start(osc[:], d_osc)
            masks.make_identity(nc, ident[:])
            if not uniform:
                vrow32 = cp.tile([1, BLOC], F32)
                delta32 = cp.tile([1, M], F32)
                vrow = cp.tile([1, BLOC], BF16)
                delta = cp.tile([1, M], BF16)
                nc.sync.dma_start(vrow32[:], d_vrow)
                nc.sync.dma_start(delta32[:], d_delta)
                nc.vector.tensor_copy(vrow[:], vrow32[:])
                nc.vector.tensor_copy(delta[:], delta32[:])

            ot0 = taccp.tile([128, BLOC], F32)
            ot1 = taccp.tile([128, BLOC], F32)

            # Software-pipelined PE stream: the w@TA matmuls of tile t-1 are
            # emitted after tile t's g/nb matmuls so the in-order PE queue
            # always has ready work while tile t's epilogue runs.
            prev = None
            for t in range(NT):
                lp = lpool.tile([128, 768 if not USE_FP8 else 384], BF16,
                                tag="lp")
                nc.sync.dma_start(lp[:], d_lp[t])
                if USE_FP8:
                    lgp = lpool.tile([64, 2, 384], F8, tag="lg")
                    nc.sync.dma_start(lgp[:], d_lg[t])
                    l3 = lp[:, 0:128]
                    ta = lp[:, 128:384]
                else:
                    l3 = lp[:, 256:384]
                    ta = lp[:, 512:768]

                G = gpool.tile([128, 1024], F32, tag="G")
                nb = npool.tile([128, BLOC], F32, tag="NB")
                if USE_FP8:
                    nc.tensor.matmul(G[:, 0:512], lgp[:, :, 0:128], r1g[:],
                                     start=True, stop=True, perf_mode=DR)
                    nc.tensor.matmul(G[:, 512:1024], lgp[:, :, 128:256], r1g[:],
                                     start=True, stop=True, perf_mode=DR)
                    nc.tensor.matmul(nb[:], l3, r1b[:],
                                     start=True, stop=False)
                    nc.tensor.matmul(nb[:], lgp[:, :, 256:384], r2g[:],
                                     start=False, stop=uniform,
                                     perf_mode=DR, skip_group_check=True)
                else:
                    nc.tensor.matmul(G[:, 0:512], lp[:, 0:128], r1g[:],
                                     start=True, stop=True)
                    nc.tensor.matmul(G[:, 512:1024], lp[:, 128:256], r1g[:],
                                     start=True, stop=True)
                    nc.tensor.matmul(nb[:], l3, r1b[:],
                                     start=True, stop=False)
                    nc.tensor.matmul(nb[:], lp[:, 384:512], r2g[:],
                                     start=False, stop=uniform,
                                     skip_group_check=True)
                if not uniform:
                    nc.tensor.matmul(nb[:], delta[:, t * 128:(t + 1) * 128],
                                     vrow[:], start=False, stop=True,
                                     skip_group_check=True)

                if prev is not None:
                    pta, pw, pt_ = prev
                    for h, oth in ((0, ot0), (1, ot1)):
                        nc.tensor.matmul(oth[:], pta[:, h * 128:(h + 1) * 128],
                                         pw[:], start=(pt_ == 0), stop=False,
                                         skip_group_check=True)

                u = upool.tile([128, 1024], FP16, tag="u")
                nc.scalar.activation(u[:, 0:AW], G[:, 0:AW], AF.Square)
                if THETA:
                    # DVE copies the tail of G to SBUF; Pool squares it there
                    gc = upool.tile([128, THETA], F32, tag="gc")
                    nc.vector.tensor_copy(gc[:], G[:, AW:1024])
                    nc.gpsimd.tensor_mul(u[:, AW:1024], gc[:], gc[:])
                t16 = upool.tile([128, BLOC], FP16, tag="t16")
                if POOL_ADD:
                    nc.gpsimd.tensor_add(t16[:, DA:512], u[:, DA:512],
                                         u[:, 512 + DA:1024])
                if DA:
                    nc.vector.tensor_add(t16[:, 0:DA], u[:, 0:DA],
                                         u[:, 512:512 + DA])
                nc.vector.scalar_tensor_tensor(
                    nb[:], t16[:], 1.0, nb[:], op0=OP.mult, op1=OP.add)
                w = wpool.tile([128, BLOC], BF16, tag="w")
                nc.scalar.activation(w[:], nb[:], AF.Exp,
                                     bias=bet[:, t:t + 1], scale=1.0)
                prev = (ta, w, t)

            pta, pw, pt_ = prev
            for h, oth in ((0, ot0), (1, ot1)):
                nc.tensor.matmul(oth[:], pta[:, h * 128:(h + 1) * 128],
                                 pw[:], start=False, stop=True,
                                 skip_group_check=True)

            # transpose out.T -> out, scaling rows by osc, then DMA out
            obs = [wpool.tile([128, S], F32, tag=f"ob{j}", bufs=1,
                              name=f"ob{j}") for j in range(4)]
            for h, oth in ((0, ot0), (1, ot1)):
                tsb = wpool.tile([128, BLOC], F32, tag="tsb")
                nc.scalar.copy(tsb[:], oth[:])
                for j in range(4):
                    pt = npool.tile([128, 128], F32, tag="NB", name=f"pt{h}{j}")
                    nc.tensor.transpose(pt[:], tsb[:, j * 128:(j + 1) * 128],
                                        ident[:])
                    nc.vector.tensor_scalar_mul(
                        obs[j][:, h * 128:(h + 1) * 128], pt[:], osc[:, j:j + 1])
            for j in range(4):
                nc.sync.dma_start(d_out[j * 128:(j + 1) * 128, :], obs[j][:])

            if POOL_DIAG:
                # throwaway Pool-timing probes (read/write scratch SBUF);
                # overlap the output phase, cost visible in the profile
                pda = wpool.tile([128, 512], FP16, tag="pda", bufs=1)
                pdb = wpool.tile([128, 512], FP16, tag="pdb", bufs=1)
                pdc = wpool.tile([128, 512], F32, tag="pdc", bufs=1)
                nc.gpsimd.tensor_add(pda[:], pdb[:], pdb[:])
                nc.gpsimd.tensor_mul(pdb[:], pda[:], pda[:])
                nc.gpsimd.tensor_add(pdc[:], pdc[:], pdc[:])
                nc.gpsimd.tensor_copy(pda[:], pdc[:])
    nc.compile()
    _CACHED[key] = nc
    return nc


def _run(inputs, trace=False):
    from concourse.bass_utils import run_bass_kernel_spmd

    prep = _prep(**inputs)
    nc = _build_nc(prep["uniform"])
    shared = {k: prep[k] for k in ("lpack", "lg", "bet", "delta")}

    in_maps = []
    for c in range(NCORES):
        sl = slice(c * BLOC, (c + 1) * BLOC)
        if USE_FP8:
            r1gc = np.ascontiguousarray(prep["r1g"][:, :, sl])
            r2gc = np.ascontiguousarray(prep["r2g"][:, :, sl])
        else:
            r1gc = np.ascontiguousarray(prep["r1g"][:, sl])
            r2gc = np.ascontiguousarray(prep["r2g"][:, sl])
        in_maps.append(dict(r1g=r1gc, r2g=r2gc,
                            r1b=np.ascontiguousarray(prep["r1b"][:, sl]),
                            vrow=np.ascontiguousarray(prep["vrow"][:, sl]),
                            osc=np.ascontiguousarray(prep["osc"][c]),
                            **shared))
    res = run_bass_kernel_spmd(nc, in_maps, list(range(NCORES)), trace=trace)
    out = np.concatenate([res.results[c]["out"] for c in range(NCORES)], 0)
    return out.astype(np.float32), res


def kernel(**inputs):
    out, _ = _run(inputs, trace=False)
    return out


def _install_ntff_hook():
    """The agent image's antenv lacks axon_hooks; recreate it so trace=True
    can capture NTFF profiles via libaxon_pjrt.so (same mechanism as
    trn_agent_boot.trn_boot)."""
    import types

    try:
        from antenv.axon_hooks import get_axon_ntff_profile_hook  # noqa: F401
        return
    except ImportError:
        pass
    import contextlib
    import ctypes

    so_path = "/opt/axon/libaxon_pjrt.so"
    lib = ctypes.CDLL(so_path)
    lib.axon_start_nrt_profile.argtypes = [ctypes.POINTER(ctypes.c_int64),
                                           ctypes.c_size_t]
    lib.axon_start_nrt_profile.restype = ctypes.c_int64
    lib.axon_stop_nrt_profile.argtypes = [ctypes.c_char_p]
    lib.axon_stop_nrt_profile.restype = ctypes.c_int64

    @contextlib.contextmanager
    def _hook(output_dir, device_ids):
        import jax

        jax.devices()
        if device_ids:
            ids = (ctypes.c_int64 * len(device_ids))(*device_ids)
            rc = lib.axon_start_nrt_profile(ids, len(device_ids))
        else:
            rc = lib.axon_start_nrt_profile(None, 0)
        if rc != 0:
            raise RuntimeError(f"axon_start_nrt_profile rc={rc}")
        try:
            yield
        finally:
            n = lib.axon_stop_nrt_profile(str(output_dir).encode())
            if n < 0:
                raise RuntimeError(f"axon_stop_nrt_profile rc={n}")
            if n == 0:
                print("WARNING: NTFF capture wrote nothing (raced the execute)")

    mod = types.ModuleType("antenv.axon_hooks")
    mod.get_axon_ntff_profile_hook = lambda: _hook
    mod.set_axon_ntff_profile_hook = lambda h: None
    sys.modules["antenv.axon_hooks"] = mod
    import antenv

    antenv.axon_hooks = mod


def run_traced(inputs):
    _install_ntff_hook()
    return _run(inputs, trace=True)
